# revision 1
# baseline (speedup 1.0000x reference)
"""BayesianKAN ECG kernel for Trainium2 (8 NeuronCores, data-parallel batch shard).

Pipeline per 128-row tile (grouped in 512-row blocks):
  x[128,1000] --DVE reduce--> pooled[128,100] --bn_stats+Newton rsqrt--> norm
  --PE transpose--> xnT[100,512] --ACT Square/Exp--> B1[100,16*512]
  --16 PE matmuls--> h[64,512] --Tanh--> --PE transpose--> norm2 (layout A)
  --PE transpose back + replicate--> B2 src[128,512] --ACT--> B2[128,8*512]
  --8 PE matmuls--> out[5,512]
Notes:
  * mean/std are scale-invariant so the pooling /10 is skipped (eps diff ~1e-6 rel).
  * ddof=1 handled by scaling biased var by n/(n-1) before rsqrt.
  * rstd = rsqrt(var) via Newton iteration w/ bit-trick seed (3 iters, ~1e-7 rel);
    reference's 1/(sd+1e-6) differs by ~1.4e-6 rel - negligible.
"""

import os
import sys
from contextlib import ExitStack

import numpy as np
import ml_dtypes

sys.path.insert(0, "/opt/trn_rl_repo")

import concourse.bass as bass
import concourse.tile as tile
from concourse import mybir
from concourse.bass_utils import run_bass_kernel_spmd


from concourse.vector_clock import ScopedClock

from concourse import bass2jax as _b2j

_orig_hook = _b2j.neuronx_cc_hook


def _dbg_hook(*a, **k):
    try:
        return _orig_hook(*a, **k)
    except BaseException:
        import traceback
        with open("/tmp/hook_err.txt", "w") as f:
            traceback.print_exc(file=f)
        raise


_b2j.neuronx_cc_hook = _dbg_hook





MAX_WAITS = 1


def _split_sync_waits(nc, limit=MAX_WAITS):
    """Walrus in this env rejects instructions with more than ~2 sync waits.

    Move excess waits onto same-engine NOPs inserted right before the
    offending instruction (in-order engines make this equivalent).
    """
    n_split = 0
    for block in nc.main_func.blocks:
        new_insts = []
        for inst in block.instructions:
            si = inst.sync_info
            waits = list(si.on_wait) if si is not None else []
            if len(waits) > limit:
                extra, keep = waits[:-limit], waits[-limit:]
                for k in range(0, len(extra), limit):
                    nop = mybir.InstNoOp(
                        name=f"{inst.name}-ws{k}",
                        sync_info=mybir.SyncInfo(
                            on_wait=extra[k : k + limit], on_update=[]
                        ),
                        bass_nofuse=True,
                        engine=inst.engine,
                    )
                    nc.register_instruction(nop, overwrite=True)
                    new_insts.append(nop)
                    n_split += 1
                si.on_wait = keep
                inst.sync_info = si
            new_insts.append(inst)
        block.instructions[:] = new_insts
    return n_split


BATCH = 65536
SEQ = 1000
IN_DIM = 100
POOLW = 10
HID = 64
OUT_DIM = 5
NB = 16
NCORES = 8
ROWS = BATCH // NCORES          # 8192 rows per core
TILE_P = 128
BLK = 512                       # rows per block
NSUB = BLK // TILE_P            # 4 sub-tiles per block
NBLK = ROWS // BLK              # 16 blocks per core

LAST_RESULTS = None

def _ensure_ntff_hook():
    """Synthesize antenv.axon_hooks (absent in this image) so trace=True works."""
    import types

    if "antenv.axon_hooks" in sys.modules:
        return
    mod = types.ModuleType("antenv.axon_hooks")
    mod._hook = None

    def set_axon_ntff_profile_hook(h):
        mod._hook = h

    def get_axon_ntff_profile_hook():
        return mod._hook

    mod.set_axon_ntff_profile_hook = set_axon_ntff_profile_hook
    mod.get_axon_ntff_profile_hook = get_axon_ntff_profile_hook
    sys.modules["antenv.axon_hooks"] = mod
    import antenv

    antenv.axon_hooks = mod
    try:
        from trn_agent_boot.trn_boot import _ntff_profile_via_ctypes

        hook = _ntff_profile_via_ctypes("/opt/axon/libaxon_pjrt.so")
        if hook is not None:
            set_axon_ntff_profile_hook(hook)
    except Exception as e:
        print("ntff hook setup failed:", e)


RBF_A = np.float32(0.5 / 0.36)  # exp(-A * d^2)
GEO_LO, GEO_HI = 4, 11          # direct slices; outer ones use the recurrence
MAGIC = 0x5F3759DF

F32 = mybir.dt.float32
BF16 = mybir.dt.bfloat16
F32R = mybir.dt.float32r
I32 = mybir.dt.int32
AF = mybir.ActivationFunctionType
ALU = mybir.AluOpType
AX = mybir.AxisListType


def _newton_rsqrt(nc, pool, var_ap, out_ap, n_par, n_free, ddof_scale):
    """out = rsqrt(var * ddof_scale) elementwise on a [n_par, n_free] AP.

    var_ap is strided fp32; Newton w/ fast-inverse-sqrt seed, 3 iterations.
    """
    v = pool.tile([n_par, n_free], F32, tag="nw_v")
    y = pool.tile([n_par, n_free], F32, tag="nw_y")
    t = pool.tile([n_par, n_free], F32, tag="nw_t")
    # v = var * ddof_scale  (also makes v contiguous)
    nc.vector.tensor_scalar(v, var_ap, float(ddof_scale), None, ALU.mult)
    # seed: y_bits = MAGIC - (v_bits >> 1)
    nc.vector.tensor_scalar(
        y.bitcast(I32), v.bitcast(I32), 1, None, ALU.logical_shift_right
    )
    nc.vector.tensor_scalar(
        y.bitcast(I32), y.bitcast(I32), -1, MAGIC, ALU.mult, ALU.add
    )
    for it in range(2):
        nc.vector.tensor_mul(t, y, y)          # t = y*y
        nc.vector.tensor_mul(t, t, v)          # t = v*y*y
        nc.vector.tensor_scalar(t, t, -0.5, 1.5, ALU.mult, ALU.add)
        nc.vector.tensor_mul(out_ap if it == 1 else y, y, t)


def build_bass(centers_np, nblk=NBLK):
    rows = nblk * BLK
    nc = bass.Bass()

    x_in = nc.declare_dram_parameter("x", [rows, SEQ], F32, isOutput=False)
    w1_in = nc.declare_dram_parameter("w1", [IN_DIM, NB * HID], F32R, isOutput=False)
    w2_in = nc.declare_dram_parameter("w2", [TILE_P, 8 * OUT_DIM], F32R, isOutput=False)
    cb1_in = nc.declare_dram_parameter("cb1", [IN_DIM, NB], F32, isOutput=False)
    gb_in = nc.declare_dram_parameter("gb", [IN_DIM, 2], F32, isOutput=False)
    cb2_in = nc.declare_dram_parameter("cb2", [TILE_P, 8], F32, isOutput=False)
    ident_in = nc.declare_dram_parameter("ident", [TILE_P, TILE_P], F32, isOutput=False)
    out_ext = nc.declare_dram_parameter("out", [OUT_DIM, rows], F32, isOutput=True)

    cvals = [float(c) for c in centers_np]
    delta = cvals[1] - cvals[0]
    # geometric recurrence for outer basis slices requires equispaced centers
    assert np.allclose(np.diff(np.asarray(centers_np, np.float64)), delta, rtol=1e-5)
    GEO_S = float(2.0 * RBF_A * delta)                 # ru scale
    GEO_BU = float(-2.0 * RBF_A * delta * cvals[GEO_HI])
    GEO_BD = float(2.0 * RBF_A * delta * cvals[GEO_LO])

    with ExitStack() as ctx:
        tc = ctx.enter_context(tile.TileContext(nc))
        singles = ctx.enter_context(tc.tile_pool(name="singles", bufs=1))
        xin_p = ctx.enter_context(tc.tile_pool(name="xin", bufs=5))
        small_p = ctx.enter_context(tc.tile_pool(name="small", bufs=4))
        nw_p = ctx.enter_context(tc.tile_pool(name="newton", bufs=2))
        xnt_p = ctx.enter_context(tc.tile_pool(name="xnt", bufs=2))
        b1_p = ctx.enter_context(tc.tile_pool(name="b1", bufs=1))
        hb_p = ctx.enter_context(tc.tile_pool(name="hb", bufs=2))
        ha_p = ctx.enter_context(tc.tile_pool(name="ha", bufs=2))
        b2s_p = ctx.enter_context(tc.tile_pool(name="b2s", bufs=2))
        geo_p = ctx.enter_context(tc.tile_pool(name="geo", bufs=2))
        xh_p = ctx.enter_context(tc.tile_pool(name="xh", bufs=3))
        b2_p = ctx.enter_context(tc.tile_pool(name="b2", bufs=1))
        ps_t = ctx.enter_context(tc.tile_pool(name="ps_t", bufs=2, space="PSUM"))
        ps_h = ctx.enter_context(tc.tile_pool(name="ps_h", bufs=1, space="PSUM"))
        ps_a = ctx.enter_context(tc.tile_pool(name="ps_a", bufs=1, space="PSUM"))
        ps_x2 = ctx.enter_context(tc.tile_pool(name="ps_x2", bufs=1, space="PSUM"))
        ps_o = ctx.enter_context(tc.tile_pool(name="ps_o", bufs=1, space="PSUM"))

        ident = singles.tile([TILE_P, TILE_P], F32)
        nc.sync.dma_start(out=ident, in_=ident_in[:, :])
        w1 = singles.tile([IN_DIM, NB, HID], F32R)
        nc.sync.dma_start(out=w1, in_=w1_in[:, :].rearrange("i (n o) -> i n o", n=NB))
        w2 = singles.tile([TILE_P, 8, OUT_DIM], F32R)
        nc.sync.dma_start(out=w2, in_=w2_in[:, :].rearrange("p (c o) -> p c o", c=8))
        cb1 = singles.tile([IN_DIM, NB], F32)
        nc.sync.dma_start(out=cb1, in_=cb1_in[:, :])
        gb = singles.tile([IN_DIM, 2], F32)
        nc.sync.dma_start(out=gb, in_=gb_in[:, :])
        cb2 = singles.tile([TILE_P, 8], F32)
        nc.sync.dma_start(out=cb2, in_=cb2_in[:, :])
        outs_p = ctx.enter_context(tc.tile_pool(name="outs", bufs=3))

        x_t = x_in[:, :].rearrange("(nb ns p) s -> nb ns p s", nb=nblk, ns=NSUB)

        def stage_a(b, xnt_pair):
            """DMA + pool + stats + norm1 + transpose -> xnt_pair half (layout B)."""
            half = (b % 2) * BLK
            mv1 = small_p.tile([TILE_P, NSUB, 2], F32, tag="mv1")
            r1 = small_p.tile([TILE_P, NSUB], F32, tag="r1")
            xp_blk = small_p.tile([TILE_P, NSUB, IN_DIM], F32, tag="xp")
            for s in range(NSUB):
                x_tile = xin_p.tile([TILE_P, SEQ], F32, tag="x")
                nc.sync.dma_start(out=x_tile, in_=x_t[b, s])
                xf = x_tile.rearrange("p (i k) -> p i k", k=POOLW)
                xh = xh_p.tile([TILE_P, IN_DIM, POOLW // 2], F32, tag="xh")
                nc.gpsimd.tensor_tensor(
                    xh, xf[:, :, 0 : POOLW // 2], xf[:, :, POOLW // 2 :], ALU.add
                )
                nc.vector.tensor_reduce(xp_blk[:, s, :], xh, AX.X, ALU.add)
                st6 = small_p.tile([TILE_P, 6], F32, tag="st6")
                nc.vector.bn_stats(st6, xp_blk[:, s, :])
                nc.vector.bn_aggr(mv1[:, s, :], st6)
            _newton_rsqrt(
                nc, nw_p, mv1[:, :, 1], r1, TILE_P, NSUB, IN_DIM / (IN_DIM - 1)
            )
            xnt_ps = ps_t.tile([IN_DIM, BLK], F32, tag="xnt_ps")
            for s in range(NSUB):
                xn = small_p.tile([TILE_P, IN_DIM], F32, tag="xn")
                nc.vector.tensor_scalar(
                    xn, xp_blk[:, s, :], mv1[:, s, 0:1], r1[:, s : s + 1],
                    ALU.subtract, ALU.mult,
                )
                nc.tensor.transpose(
                    xnt_ps[:, s * TILE_P : (s + 1) * TILE_P], xn, ident
                )
            nc.vector.tensor_copy(xnt_pair[:, half : half + BLK], xnt_ps)

        def stage_l1(bp, xnt):
            """Pair-batched basis1 + mm1 + tanh -> hb."""
            PBLK = 2 * BLK
            b1 = b1_p.tile([IN_DIM, NB, PBLK], F32R, tag="b1")
            for n in range(GEO_LO, GEO_HI + 1):
                nc.scalar.activation(b1[:, n, :], xnt, AF.Square, bias=cb1[:, n : n + 1])
            nc.scalar.activation(
                b1[:, GEO_LO : GEO_HI + 1, :].rearrange("i n k -> i (n k)"),
                b1[:, GEO_LO : GEO_HI + 1, :].rearrange("i n k -> i (n k)"),
                AF.Exp,
                scale=float(-RBF_A),
            )
            # outer slices via exact ratio recurrence:
            # B_{n+1} = B_n * exp(2*A*D*(x - c_n) - A*D^2), const part folded
            # into w1 on the host; ru/rd = exp(+-2*A*D*(x - c_anchor)).
            ru = geo_p.tile([IN_DIM, PBLK], F32R, tag="ru")
            rd = geo_p.tile([IN_DIM, PBLK], F32R, tag="rd")
            nc.scalar.activation(ru, xnt, AF.Exp, scale=GEO_S, bias=gb[:, 0:1])
            nc.scalar.activation(rd, xnt, AF.Exp, scale=-GEO_S, bias=gb[:, 1:2])
            for n in range(GEO_HI + 1, NB):
                nc.vector.tensor_mul(b1[:, n, :], b1[:, n - 1, :], ru)
            for n in range(GEO_LO - 1, -1, -1):
                nc.vector.tensor_mul(b1[:, n, :], b1[:, n + 1, :], rd)
            h_ps = ps_h.tile([HID, PBLK], F32, tag="h_ps")
            mm_order = list(range(GEO_LO, GEO_HI + 1)) + [12, 3, 13, 2, 14, 1, 15, 0]
            for mi, n in enumerate(mm_order):
                for hf in range(2):
                    nc.tensor.matmul(
                        h_ps[:, hf * BLK : (hf + 1) * BLK],
                        w1[:, n, :],
                        b1[:, n, hf * BLK : (hf + 1) * BLK],
                        start=(mi == 0),
                        stop=(mi == len(mm_order) - 1),
                    )
            hb = hb_p.tile([HID, PBLK], F32, tag="hb")
            nc.scalar.activation(hb, h_ps, AF.Tanh)
            return hb

        def stage_l2(bp, hb):
            """norm2 + basis2 + mm2 + out for pair bp."""
            PBLK = 2 * BLK
            # layout-A roundtrip for norm2 over 8 sub-tiles of the pair
            ha_ps = ps_a.tile([TILE_P, 8 * HID], F32, tag="ha_ps")
            for s in range(8):
                nc.tensor.transpose(
                    ha_ps[:, s * HID : (s + 1) * HID],
                    hb[:, s * TILE_P : (s + 1) * TILE_P],
                    ident[:HID, :HID],
                )
            ha = ha_p.tile([TILE_P, 8, HID], F32, tag="ha")
            nc.vector.tensor_copy(ha.rearrange("p s h -> p (s h)"), ha_ps)
            mv2 = small_p.tile([TILE_P, 8, 2], F32, tag="mv2")
            r2 = small_p.tile([TILE_P, 8], F32, tag="r2")
            for s in range(8):
                st6b = small_p.tile([TILE_P, 6], F32, tag="st6b")
                nc.vector.bn_stats(st6b, ha[:, s, :])
                nc.vector.bn_aggr(mv2[:, s, :], st6b)
            _newton_rsqrt(nc, nw_p, mv2[:, :, 1], r2, TILE_P, 8, HID / (HID - 1))

            xn2_ps = ps_x2.tile([HID, PBLK], F32, tag="xn2_ps")
            for s in range(8):
                hn = small_p.tile([TILE_P, HID], F32, tag="hn")
                nc.vector.tensor_scalar(
                    hn, ha[:, s, :], mv2[:, s, 0:1], r2[:, s : s + 1],
                    ALU.subtract, ALU.mult,
                )
                nc.tensor.transpose(
                    xn2_ps[:, s * TILE_P : (s + 1) * TILE_P], hn, ident
                )
            b2s = b2s_p.tile([TILE_P, PBLK], F32, tag="b2s")
            nc.vector.tensor_copy(b2s[:HID, :], xn2_ps)
            # cross-partition replicate (lanes are partition-locked -> DMA)
            nc.sync.dma_start(out=b2s[HID:, :], in_=b2s[:HID, :])

            b2 = b2_p.tile([TILE_P, 8, PBLK], F32R, tag="b2")
            for c in range(8):
                nc.scalar.activation(
                    b2[:, c, :], b2s, AF.Square, bias=cb2[:, c : c + 1]
                )
            nc.scalar.activation(
                b2.rearrange("p c k -> p (c k)"),
                b2.rearrange("p c k -> p (c k)"),
                AF.Exp,
                scale=float(-RBF_A),
            )
            for hf in range(2):
                o_ps = ps_o.tile([OUT_DIM, BLK], F32, tag="o_ps")
                for c in range(8):
                    nc.tensor.matmul(
                        o_ps,
                        w2[:, c, :],
                        b2[:, c, hf * BLK : (hf + 1) * BLK],
                        start=(c == 0),
                        stop=(c == 7),
                    )
                out_sb = outs_p.tile([OUT_DIM, BLK], F32, tag="out_sb")
                nc.vector.tensor_copy(out_sb, o_ps)
                bb = 2 * bp + hf
                nc.sync.dma_start(
                    out=out_ext[:, bb * BLK : (bb + 1) * BLK], in_=out_sb
                )

        LEAD = 0  # L1 fires right after its pair; L2 runs one pair behind L1
        xnts = {}
        hbs = {}
        npairs = nblk // 2
        for b in range(nblk + LEAD + 2):
            if b < nblk:
                pair = b // 2
                if b % 2 == 0:
                    xnt_pair_t = xnt_p.tile([IN_DIM, 2 * BLK], F32, tag="xnt")
                    xnts[pair] = xnt_pair_t
                stage_a(b, xnts[pair])
            done = b - LEAD
            if done >= 0 and done % 2 == 1 and done // 2 < npairs:
                p1 = done // 2
                hbs[p1] = stage_l1(p1, xnts.pop(p1))
                if p1 >= 1:
                    stage_l2(p1 - 1, hbs.pop(p1 - 1))
        stage_l2(npairs - 1, hbs.pop(npairs - 1))

    _split_sync_waits(nc)
    return nc


def _host_consts(c1_mu, c2_mu, centers):
    # w1[i, n*HID+o] = c1_mu[o, i, n]
    w1f = np.transpose(c1_mu, (1, 2, 0)).astype(np.float64).copy()  # [i, n, o]
    delta = float(centers[1] - centers[0])
    a = float(RBF_A)
    for n in range(NB):
        if n > GEO_HI:
            k = n - GEO_HI
        elif n < GEO_LO:
            k = GEO_LO - n
        else:
            continue
        w1f[:, n, :] *= np.exp(-a * delta * delta * k * k)
    w1 = np.ascontiguousarray(w1f.reshape(IN_DIM, NB * HID)).astype(np.float32)
    # B2 partition p (of chunk c) holds i = p % 64, n = 2c + p // 64
    w2 = np.zeros((TILE_P, 8, OUT_DIM), np.float32)
    cb2 = np.zeros((TILE_P, 8), np.float32)
    for p in range(TILE_P):
        i = p % HID
        for c in range(8):
            n = 2 * c + p // HID
            w2[p, c, :] = c2_mu[:, i, n]
            cb2[p, c] = -centers[n]
    w2 = w2.reshape(TILE_P, 8 * OUT_DIM)
    cb1 = np.tile(-centers[None, :], (IN_DIM, 1)).astype(np.float32)
    gbu = -2.0 * a * delta * float(centers[GEO_HI])
    gbd = 2.0 * a * delta * float(centers[GEO_LO])
    gb = np.tile(np.array([[gbu, gbd]], np.float32), (IN_DIM, 1))
    ident = np.eye(TILE_P, dtype=np.float32)
    return w1, w2, cb1, cb2, gb, ident


def kernel(x, c1_mu, c2_mu, centers):
    x = np.asarray(x, np.float32)
    batch = x.shape[0]
    rows = batch // NCORES
    nblk = rows // BLK
    assert rows % BLK == 0
    c1_mu = np.asarray(c1_mu, np.float32)
    c2_mu = np.asarray(c2_mu, np.float32)
    centers = np.asarray(centers, np.float32)

    w1, w2, cb1, cb2, gb, ident = _host_consts(c1_mu, c2_mu, centers)
    nc = build_bass(centers, nblk)

    in_maps = []
    for i in range(NCORES):
        in_maps.append(
            {
                "x": np.ascontiguousarray(x[i * rows : (i + 1) * rows]),
                "w1": w1,
                "w2": w2,
                "cb1": cb1,
                "cb2": cb2,
                "gb": gb,
                "ident": ident,
            }
        )
    trace = bool(int(os.environ.get("BASS_KERNEL_TRACE", "0")))
    if trace:
        sys.path.insert(0, "/root/.axon_site")
        _ensure_ntff_hook()
    res = run_bass_kernel_spmd(
        nc, in_maps, list(range(NCORES)), trace=trace
    )
    global LAST_RESULTS
    LAST_RESULTS = res
    out = np.empty((batch, OUT_DIM), np.float32)
    for i in range(NCORES):
        out[i * rows : (i + 1) * rows] = res.results[i]["out"].T
    return out


if __name__ == "__main__":
    xs = np.random.randn(BATCH, SEQ).astype(np.float32)
    c1 = (np.random.randn(HID, IN_DIM, NB) * 0.05).astype(np.float32)
    c2 = (np.random.randn(OUT_DIM, HID, NB) * 0.05).astype(np.float32)
    cen = np.linspace(-3, 3, NB).astype(np.float32)
    print(kernel(xs, c1, c2, cen)[:2])

